# revision 9
# baseline (speedup 1.0000x reference)
"""Trainium2 Bass kernel: AttentionBlock (B=2, S=2048, D=1024, H=16) on 8 cores.

Head-sharded design (tensor parallel over heads, data parallel over batch):
core c -> batch b=c//4, group rank g=c%4; owns heads [4g, 4g+4) of its batch.
Each core computes K/V/Q for its 4 own heads over ALL 2048 tokens locally --
NO collective before attention. Attention runs over (token-quarter, head)
segments; after each token quarter's 4 heads finish, the out-projection
partial (contraction over the core's 256 features) is computed and a
bf16 ReduceScatter(add) over the 4-core group sums the partials and hands
each core a 128-token block; the residual x (with bp host-folded in) is added
post-RS so y = x + proj + bp. LayerNorm runs post-RS in token-on-partition
orientation with STT-accumulator sums and a DVE-only bit-trick rsqrt (this
runtime's tensor_reduce crashes the DVE, and an ACT Sqrt would head-of-line
block the exp stream behind the collective).

Precision: fp8e4m3 for all projection inputs (x, Wq/Wk/Wv/Wp, ot) using
DoubleRow matmuls (2x PE throughput), fp8 softmax weights (exp output with a
-3 shift so e^s fits fp8 range; softmax is shift-invariant), bf16 K/Q for
scores, bf16 RS. Measured end-to-end rel err ~4.5e-3 in numpy emulation
(tolerance 2e-2).

ScalarE (exp) is the pacing engine: 128 merged exp-pairs x ~1.04us. The PE
stream (scores + PV-DoubleRow + projection fillers) runs ~90us against
ScalarE's ~133us, so scores stay ahead of exp and PV trails it.
"""

import sys

for _p in (
    "/root/.axon_site",
    "/root/.axon_site/_ro/trn_rl_repo",
    "/root/.axon_site/_ro/pypackages",
    "/opt/trn_rl_repo",
    "/opt/pypackages",
):
    if _p not in sys.path:
        sys.path.append(_p)

import numpy as np
import ml_dtypes
from contextlib import ExitStack

import concourse.bass as bass
import concourse.mybir as mybir
import concourse.tile as tile
from concourse import bacc
from concourse.bass_utils import run_bass_kernel_spmd

F32 = mybir.dt.float32
BF16 = mybir.dt.bfloat16
FP8 = mybir.dt.float8e4
BF16_NP = ml_dtypes.bfloat16
FP8_NP = ml_dtypes.float8_e4m3
AF = mybir.ActivationFunctionType
ALU = mybir.AluOpType
DR = mybir.MatmulPerfMode.DoubleRow

P = 128
B = 2
S = 2048          # tokens per batch
D = 1024
H = 16
HD = 64
TOK = 512         # tokens per attention quarter-segment (and out tokens/core)
DC = D // P       # 8 feature chunks
KC = S // P       # 16 key chunks
NPAIR = KC // 2   # 8 exp-merged key-chunk pairs
NQ = 4            # token quarters
NH = 4            # own heads per core
NCORES = 8
GSIZE = 4
DF_OWN = 256      # own head-dim features per core (4 heads x 64)
FC = DF_OWN // P  # 2 own-feature chunks
EPS = 1e-5
RG = [[0, 1, 2, 3], [4, 5, 6, 7]]
SCALE = 1.0 / np.sqrt(HD)
ESHIFT = -3.0     # exp(s*SCALE + ESHIFT): keeps e^s in fp8 range
TB = TOK // GSIZE  # 128-token RS output block


def _body(nc, tc, io, profile=False):
    (xt, xres, wkvq, wp, bqk, brep, out_ext) = io

    with ExitStack() as ctx:
        consts = ctx.enter_context(tc.tile_pool(name="consts", bufs=1))
        sc_ps = ctx.enter_context(tc.tile_pool(name="sc_ps", bufs=2, space="PSUM"))
        mm_ps = ctx.enter_context(tc.tile_pool(name="mm_ps", bufs=2, space="PSUM"))
        o_ps = ctx.enter_context(tc.tile_pool(name="o_ps", bufs=2, space="PSUM"))
        ptp = ctx.enter_context(tc.tile_pool(name="ptp", bufs=4))
        small = ctx.enter_context(tc.tile_pool(name="small", bufs=1))
        repp = ctx.enter_context(tc.tile_pool(name="repp", bufs=2))
        stg = ctx.enter_context(tc.tile_pool(name="stg", bufs=4))
        post = ctx.enter_context(tc.tile_pool(name="post", bufs=2))
        dram = ctx.enter_context(tc.tile_pool(name="dram", bufs=1, space="DRAM"))

        # ---------------- input loads (SP/HWDGE queue, consumption order) ----
        # DMA_ENGINES is effectively serial: only attention-critical tensors
        # go before the remaining xt quarters; wp/xres/brep are deferred.
        xt_sb = consts.tile([P, DC, S], FP8)
        xt_r = xt.rearrange("(c p) t -> p c t", p=P)
        # wkvq: host-packed [128, 3, 8, 256] (2KB/partition contiguous rows);
        # loaded in three slices so wk lands before xt quarter 0 finishes
        wkvq_sb = consts.tile([P, 3, DC, DF_OWN], FP8)
        wk_sb = wkvq_sb[:, 0]
        wv_sb = wkvq_sb[:, 1]
        wq_sb = wkvq_sb[:, 2]
        nc.sync.dma_start(wkvq_sb[:, 0], wkvq[:, 0])
        nc.sync.dma_start(xt_sb[:, :, 0:TOK], xt_r[:, :, 0:TOK])
        bqk_sb = consts.tile([P, 6], F32)   # bq(2) bk(2) bv(2)
        nc.sync.dma_start(bqk_sb[:], bqk)
        nc.sync.dma_start(wkvq_sb[:, 2], wkvq[:, 2])
        nc.sync.dma_start(wkvq_sb[:, 1], wkvq[:, 1])
        for q in range(1, NQ):
            nc.sync.dma_start(xt_sb[:, :, q * TOK:(q + 1) * TOK],
                              xt_r[:, :, q * TOK:(q + 1) * TOK])
        wp_sb = consts.tile([P, FC, D], FP8)
        xres_sb = consts.tile([P, NQ, D], BF16)   # x residual blocks per quarter
        brep_sb = consts.tile([P, 3, D], BF16)    # (unused), g-rep, lnb-rep

        # ---------------- on-chip tensors ----------------
        k_sb = consts.tile([P, FC, S], BF16)          # [d(2 heads/chunk), keys]
        q_sb = consts.tile([P, FC, NQ, TOK], BF16)    # [d, tq, tokens]
        # [keys, kc-pair, head, pair-slot, d+ones(+pad)]: the DoubleRow
        # weights AP needs the k-tile step to be a multiple of 16, so each
        # (pair-slot) stride is padded to 80 elements
        VS = 80
        v_sb = consts.tile([P, NPAIR, NH, 2, VS], FP8)
        nc.vector.memset(v_sb[:, :, :, :, HD:HD + 1], 1.0)
        esh_sb = consts.tile([P, 1], F32)
        nc.vector.memset(esh_sb[:], ESHIFT)
        eps_sb = consts.tile([P, 1], F32)
        nc.vector.memset(eps_sb[:], EPS)
        ot_sb = consts.tile([P, FC, S], FP8)          # [ownf, tokens]

        # RS buffers (DRAM)
        parts = [dram.tile([TOK, D], BF16, name=f"part{q}") for q in range(NQ)]
        youts = [dram.tile([TB, D], BF16, name=f"yout{q}") for q in range(NQ)]

        # ---------------- PE filler groups (projections) ----------------
        # Each filler emits a handful of matmuls + a DVE copy-out; they are
        # popped into the scores/PV stream so PE never outruns ScalarE and
        # K/V/Q land just before the attention segments need them.
        def k_proj(t4, fcs=(0, 1)):  # keys t4*512 .. +512
            for fc in fcs:
                ps = mm_ps.tile([P, TOK], F32, tag="mm", name="ps_k")
                for g in range(4):
                    nc.tensor.matmul(
                        ps[:],
                        lhsT=wk_sb[:, 2 * g:2 * g + 2, fc * P:(fc + 1) * P],
                        rhs=xt_sb[:, 2 * g:2 * g + 2, t4 * TOK:(t4 + 1) * TOK],
                        start=(g == 0), stop=(g == 3), perf_mode=DR,
                    )
                tsl = slice(t4 * TOK, (t4 + 1) * TOK)
                nc.vector.tensor_scalar_add(
                    k_sb[:, fc, tsl], ps[:], bqk_sb[:, 2 + fc:3 + fc])

        def q_proj(t4, fcs=(0, 1)):
            for fc in fcs:
                ps = mm_ps.tile([P, TOK], F32, tag="mm", name="ps_q")
                for g in range(4):
                    nc.tensor.matmul(
                        ps[:],
                        lhsT=wq_sb[:, 2 * g:2 * g + 2, fc * P:(fc + 1) * P],
                        rhs=xt_sb[:, 2 * g:2 * g + 2, t4 * TOK:(t4 + 1) * TOK],
                        start=(g == 0), stop=(g == 3), perf_mode=DR,
                    )
                nc.vector.tensor_scalar_add(
                    q_sb[:, fc, t4, :], ps[:], bqk_sb[:, fc:fc + 1])

        def v_proj(t16):  # keys t16*128 .. +128 -> v_sb[:, t16, :, :]
            ps = mm_ps.tile([P, TOK], F32, tag="mm", name="ps_v")
            for g in range(4):
                nc.tensor.matmul(
                    ps[:, :DF_OWN],
                    lhsT=xt_sb[:, 2 * g:2 * g + 2, t16 * P:(t16 + 1) * P],
                    rhs=wv_sb[:, 2 * g:2 * g + 2, :],
                    start=(g == 0), stop=(g == 3), perf_mode=DR,
                )
            # [tok, 256] -> strided [tok, head, 0:64]; bv added post-PV
            nc.vector.tensor_copy(
                v_sb[:, t16 // 2, :, t16 % 2, 0:HD],
                ps[:, :DF_OWN].rearrange("p (h d) -> p h d", h=NH))

        def out_proj(tq, tc):
            # partial[tok-chunk, :] for token quarter tq; bp/4 folded into
            # every rank's partial (sums to bp across the RS). The two
            # feature-half copies split over DVE/Pool to avoid serializing;
            # Pool is ~1.7x slower, so it gets every other half at most.
            t0 = tq * TOK + tc * P
            st = stg.tile([P, D], BF16, tag="stg", name="stg")
            for fh in range(2):
                ps = mm_ps.tile([P, TOK], F32, tag="mm", name="ps_p")
                nc.tensor.matmul(
                    ps[:, :D // 2],
                    lhsT=ot_sb[:, :, t0:t0 + P],
                    rhs=wp_sb[:, :, fh * (D // 2):(fh + 1) * (D // 2)],
                    start=True, stop=True, perf_mode=DR,
                )
                fsl = slice(fh * (D // 2), (fh + 1) * (D // 2))
                # bp is folded into xres on the host, so staging is a plain
                # psum->bf16 copy; the tail quarter's copies run on ScalarE
                # (idle after the last exp) to shorten the serial DVE tail
                if tq == NQ - 1 and (tc + fh) % 2 == 0:
                    nc.scalar.activation(st[:, fsl], ps[:, :D // 2], AF.Copy)
                else:
                    nc.vector.tensor_copy(st[:, fsl], ps[:, :D // 2])
            nc.sync.dma_start(parts[tq][tc * P:(tc + 1) * P, :], st[:])

        # K (all chunks), first V chunks, and Q(quarter 0) must be emitted
        # BEFORE the first scores matmul: the PE queue executes in order, so a
        # score emitted ahead of the projection it reads would deadlock.
        # PE warmup: garbage matmuls during the DMA wait so the projection
        # chain starts at full p-state instead of 0.65GHz
        warm_sb = consts.tile([P, TOK], BF16)
        nc.vector.memset(warm_sb[:], 0.0)
        for _ in range(10):
            wps = mm_ps.tile([P, TOK], F32, tag="mm", name="warm")
            nc.tensor.matmul(wps[0:1, :], lhsT=warm_sb[:, 0:1], rhs=warm_sb[:],
                             start=True, stop=True)
        # first scores only need K/Q chunk fc=0 of quarter 0: emit those first
        # so ScalarE starts ~6us earlier; V trails (PV needs it ~1.5us later)
        k_proj(0, fcs=(0,)); q_proj(0, fcs=(0,))
        k_proj(0, fcs=(1,)); q_proj(0, fcs=(1,))
        v_proj(0); v_proj(1)
        k_proj(1); v_proj(2); v_proj(3)
        k_proj(2); k_proj(3)
        # deferred loads: not needed until the first out_proj/post_rs (~45us+)
        nc.sync.dma_start(wp_sb[:], wp.rearrange("(c p) n -> p c n", p=P))
        nc.sync.dma_start(xres_sb[:], xres)
        nc.sync.dma_start(brep_sb[:], brep)
        fillers = []
        for t16 in range(4, KC):
            fillers.append(lambda t16=t16: v_proj(t16))
        for t4 in range(1, NQ):
            fillers.append(lambda t4=t4, fc=0: q_proj(t4, fcs=(0,)))
            fillers.append(lambda t4=t4, fc=1: q_proj(t4, fcs=(1,)))

        pending = []   # paced out-proj / RS-trigger work, one item per slot:
                       # emitting them as a burst head-of-line blocks the PE
                       # queue on the DVE staging TTs via the mm_ps rotation

        def pop_filler(n=2):
            if pending:
                pending.pop(0)()
                return
            for _ in range(n):
                if fillers:
                    fillers.pop(0)()

        # ---------------- RS + post-RS (LayerNorm on token-partition) -------
        def trigger_rs(tq):
            if profile:
                pass
            else:
                nc.gpsimd.collective_compute(
                    "ReduceScatter", ALU.add, replica_groups=RG,
                    ins=[parts[tq].opt()], outs=[youts[tq].opt()],
                )

        def post_rs(tq, par=False):
            yr = post.tile([P, D], BF16, tag="yr", name="yr")
            nc.sync.dma_start(yr[:], parts[tq][0:TB, :] if profile
                              else youts[tq][:])
            # residual add (this core's token block) with the token-sum fused in
            y = post.tile([P, D], BF16, tag="y", name="y")
            ssum = small.tile([P, 1], F32, tag="ssum", name="ssum", bufs=2)
            ssq = small.tile([P, 1], F32, tag="ssq", name="ssq", bufs=2)
            nc.vector.scalar_tensor_tensor(y[:], yr[:], 1.0, xres_sb[:, tq, :],
                                           ALU.mult, ALU.add, accum_out=ssum[:])
            # sum(y^2) via a second STT with accumulator (tensor_reduce and
            # tensor_tensor_reduce crash this runtime's DVE)
            ysc = post.tile([P, D], BF16, tag="ysc", name="ysc")
            nc.vector.scalar_tensor_tensor(ysc[:], y[:], 1.0, y[:],
                                           ALU.mult, ALU.mult, accum_out=ssq[:])
            mean = small.tile([P, 1], F32, tag="mean", name="mean", bufs=2)
            nc.vector.tensor_scalar_mul(mean[:], ssum[:], 1.0 / D)
            var = small.tile([P, 1], F32, tag="var", name="var", bufs=2)
            nc.vector.tensor_mul(var[:], mean[:], mean[:])
            ex2 = small.tile([P, 1], F32, tag="ex2", name="ex2", bufs=2)
            nc.vector.tensor_scalar(ex2[:], ssq[:], 1.0 / D, EPS,
                                    ALU.mult, ALU.add)
            nc.vector.tensor_sub(var[:], ex2[:], var[:])
            # invstd = rsqrt(var) entirely on DVE (bit-trick + 2 Newton
            # iterations): an ACT Sqrt here would sit blocked on the RS in
            # the ScalarE queue and head-of-line stall the exp stream
            I32 = mybir.dt.int32
            xg = small.tile([P, 1], F32, tag="xg", name="xg", bufs=2)
            nc.vector.tensor_scalar(xg[:].bitcast(I32), var[:].bitcast(I32),
                                    1, None, ALU.arith_shift_right)
            nc.vector.tensor_scalar(xg[:].bitcast(I32), xg[:].bitcast(I32),
                                    -1, 0x5F3759DF, ALU.mult, ALU.add)
            tnw = small.tile([P, 1], F32, tag="tnw", name="tnw", bufs=2)
            for _ in range(2):
                nc.vector.tensor_mul(tnw[:], var[:], xg[:])
                nc.vector.tensor_mul(tnw[:], tnw[:], xg[:])
                nc.vector.tensor_scalar(tnw[:], tnw[:], -0.5, 1.5,
                                        ALU.mult, ALU.add)
                nc.vector.tensor_mul(xg[:], xg[:], tnw[:])
            t1 = post.tile([P, D], BF16, tag="t1", name="t1")
            t2 = post.tile([P, D], BF16, tag="t2", name="t2")
            yo = post.tile([P, D], BF16, tag="yo", name="yo")
            # par=True: whole apply on Pool so it runs concurrently with the
            # next post_rs chain on DVE (tail has two chains back to back)
            eng = nc.gpsimd if par else nc.vector
            eng.tensor_scalar(t1[:], y[:], mean[:], xg[:],
                              ALU.subtract, ALU.mult)
            if par or tq == NQ - 1:
                eng.tensor_mul(t2[:], t1[:], brep_sb[:, 1, :])
            else:
                nc.gpsimd.tensor_mul(t2[:], t1[:], brep_sb[:, 1, :])
            eng.tensor_tensor(yo[:], t2[:], brep_sb[:, 2, :], ALU.add)
            nc.sync.dma_start(out_ext[tq * TB:(tq + 1) * TB, :], yo[:])

        # ---------------- attention segments ----------------
        def seg_epilogue(tq, h):
            # po -> ot (normalize + bv)
            po = seg_po[(tq, h)]
            fc, off = h // 2, (h % 2) * HD
            rden = small.tile([1, TOK], F32, tag="rden", name="rden", bufs=2)
            nc.vector.reciprocal(rden[:], po[HD:HD + 1, :])
            rep = repp.tile([HD, TOK], F32, tag="rep", name="rep")
            nc.gpsimd.partition_broadcast(rep[:], rden[:])
            tsl = slice(tq * TOK, (tq + 1) * TOK)
            onorm = stg.tile([HD, TOK], F32, tag="onorm", name="onorm")
            nc.vector.tensor_tensor(onorm[:], po[0:HD, :], rep[:], ALU.mult)
            nc.vector.tensor_scalar_add(
                ot_sb[off:off + HD, fc, tsl], onorm[:],
                bqk_sb[off:off + HD, 4 + fc:5 + fc])

        seg_po = {}
        segs = [(tq, h) for tq in range(NQ) for h in range(NH)]
        for si, (tq, h) in enumerate(segs):
            fc, off = h // 2, (h % 2) * HD
            po = o_ps.tile([HD + 1, TOK], F32, tag="o", name="po")
            seg_po[(tq, h)] = po
            prev_pt = None
            for pr in range(NPAIR):
                ps2 = sc_ps.tile([P, 2, TOK], F32, tag="sc", name="ps_s")
                for j in range(2):
                    kcs = slice((2 * pr + j) * P, (2 * pr + j + 1) * P)
                    nc.tensor.matmul(
                        ps2[:, j, :],
                        lhsT=k_sb[off:off + HD, fc, kcs],
                        rhs=q_sb[off:off + HD, fc, tq, :],
                        start=True, stop=True,
                    )
                if pr > 0:
                    nc.tensor.matmul(
                        po[:], lhsT=v_sb[:, pr - 1, h, :, 0:HD + 1],
                        rhs=prev_pt[:], start=(pr == 1), stop=False,
                        perf_mode=DR, skip_group_check=True,
                    )
                elif si > 0:
                    ptq, ph = segs[si - 1]
                    nc.tensor.matmul(
                        seg_po[(ptq, ph)][:],
                        lhsT=v_sb[:, NPAIR - 1, ph, :, 0:HD + 1],
                        rhs=prev_seg_pt[:],
                        start=False, stop=True, perf_mode=DR,
                        skip_group_check=True,
                    )
                    seg_epilogue(ptq, ph)
                    if h == 0 and tq >= 1:
                        # previous quarter's ot complete -> partial + RS,
                        # paced one per pair slot
                        for tkc in range(NQ):
                            pending.append(
                                lambda tq=tq, tkc=tkc: out_proj(tq - 1, tkc))
                        pending.append(lambda tq=tq: trigger_rs(tq - 1))
                    if h == 2 and tq >= 2:
                        # deprioritized: the tile scheduler otherwise hoists
                        # these RS-gated ops ahead of ready work in the SP/DVE
                        # streams and head-of-line blocks the window
                        with tc.high_priority(offset=-100000):
                            post_rs(tq - 2)
                pop_filler()
                pt2 = ptp.tile([P, 2, TOK], FP8, tag="pt", name="pt")
                nc.scalar.activation(pt2[:], ps2[:], AF.Exp,
                                     scale=float(SCALE), bias=esh_sb[:])
                prev_pt = pt2
            prev_seg_pt = prev_pt

        # tail: last segment's PV + epilogue + last quarter partial/RS/post
        tq, h = segs[-1]
        nc.tensor.matmul(
            seg_po[(tq, h)][:], lhsT=v_sb[:, NPAIR - 1, h, :, 0:HD + 1],
            rhs=prev_seg_pt[:], start=False, stop=True, perf_mode=DR,
            skip_group_check=True,
        )
        seg_epilogue(tq, h)
        for tkc in range(NQ):
            out_proj(NQ - 1, tkc)
        trigger_rs(NQ - 1)
        with tc.high_priority(offset=-100000):
            post_rs(NQ - 2)
            post_rs(NQ - 1)


def build(profile=False):
    try:
        from concourse.bass_utils import axon_active
        debug = not axon_active()
    except Exception:
        debug = False
    nc = bacc.Bacc(
        "TRN2", target_bir_lowering=False, debug=debug,
        num_devices=1 if profile else NCORES,
    )
    xt = nc.dram_tensor("xt", [D, S], FP8, kind="ExternalInput")
    xres = nc.dram_tensor("xres", [P, NQ, D], BF16, kind="ExternalInput")
    wkvq = nc.dram_tensor("wkvq", [P, 3, DC, DF_OWN], FP8, kind="ExternalInput")
    wp = nc.dram_tensor("wp", [DF_OWN, D], FP8, kind="ExternalInput")
    bqk = nc.dram_tensor("bqk", [P, 6], F32, kind="ExternalInput")
    brep = nc.dram_tensor("brep", [P, 3, D], BF16, kind="ExternalInput")
    out_ext = nc.dram_tensor("out", [TOK, D], BF16, kind="ExternalOutput")

    io = (xt[:], xres[:], wkvq[:], wp[:], bqk[:], brep[:], out_ext[:])
    with tile.TileContext(nc) as tc:
        _body(nc, tc, io, profile=profile)
    nc.compile()
    return nc


_NC = None


def _get_nc():
    global _NC
    if _NC is None:
        _NC = build()
    return _NC


def shard_inputs(inputs):
    x = np.asarray(inputs["x"], np.float32)
    Wq = np.asarray(inputs["Wq"], np.float32)
    Wk = np.asarray(inputs["Wk"], np.float32)
    Wv = np.asarray(inputs["Wv"], np.float32)
    Wp = np.asarray(inputs["Wp"], np.float32)
    bq = np.asarray(inputs["bq"], np.float32)
    bk = np.asarray(inputs["bk"], np.float32)
    bv = np.asarray(inputs["bv"], np.float32)
    bp = np.asarray(inputs["bp"], np.float32)
    lng = np.asarray(inputs["ln_g"], np.float32)
    lnb = np.asarray(inputs["ln_b"], np.float32)

    xt_b = [np.ascontiguousarray(x[b].T).astype(FP8_NP) for b in range(B)]

    in_maps = []
    for c in range(NCORES):
        b, g = c // GSIZE, c % GSIZE
        own = slice(g * DF_OWN, (g + 1) * DF_OWN)
        # residual blocks: quarter q -> tokens q*512 + g*128 .. +128
        xres = np.stack(
            [x[b, q * TOK + g * TB:q * TOK + (g + 1) * TB, :] + bp
             for q in range(NQ)], axis=1)          # [128, 4, 1024]
        wkvq_pack = np.ascontiguousarray(
            np.stack([Wk[:, own], Wv[:, own], Wq[:, own]])   # [3, 1024, 256]
            .reshape(3, DC, P, DF_OWN).transpose(2, 0, 1, 3))  # [128, 3, 8, 256]
        bqk_pack = np.stack([
            bq[own][0:P], bq[own][P:2 * P],
            bk[own][0:P], bk[own][P:2 * P],
            bv[own][0:P], bv[own][P:2 * P],
        ], axis=1)                                  # [128, 6]
        brep_pack = np.stack([
            np.tile(bp / 4, (P, 1)), np.tile(lng, (P, 1)), np.tile(lnb, (P, 1)),
        ], axis=1)                                  # [128, 3, 1024]
        in_maps.append({
            "xt": xt_b[b],
            "xres": np.ascontiguousarray(xres).astype(BF16_NP),
            "wkvq": wkvq_pack.astype(FP8_NP),
            "wp": np.ascontiguousarray(Wp[own, :]).astype(FP8_NP),
            "bqk": np.ascontiguousarray(bqk_pack),
            "brep": np.ascontiguousarray(brep_pack).astype(BF16_NP),
        })
    return in_maps


def assemble(results):
    out = np.empty((B, S, D), np.float32)
    for c in range(NCORES):
        b, g = c // GSIZE, c % GSIZE
        r = np.asarray(results[c]["out"], np.float32)  # [512, 1024]
        for q in range(NQ):
            out[b, q * TOK + g * TB:q * TOK + (g + 1) * TB, :] = \
                r[q * TB:(q + 1) * TB, :]
    return out


def run(inputs, trace=False):
    nc = _get_nc()
    in_maps = shard_inputs(inputs)
    res = run_bass_kernel_spmd(nc, in_maps, core_ids=list(range(NCORES)), trace=trace)
    return assemble(res.results), res.exec_time_ns


def kernel(**inputs):
    out, _ = run(inputs)
    return out
